# revision 8
# baseline (speedup 1.0000x reference)
"""Trainium2 Bass kernel for nn_ExpertsChooseMaskedExpand (MoE routing).

Reference computes (per batch b):
    xd[e,c,j] = sum_t mask[t,e,c] * x[t,e,j]          (dispatch)
    y[e,c,o]  = sum_j xd[e,c,j] * w[e,o,j] + bias[o]  (expert GEMM)
    out[t,o]  = sum_{e,c} comb[t,e,c] * y[e,c,o]      (combine)

We use associativity to contract comb with xd first:
    z[t,e,j] = sum_c comb[t,e,c] * xd[e,c,j]
    out[t,o] = sum_{e,j} z[t,e,j] * w[e,o,j] + bias[o] * S[t],
    S[t] = sum_{e,c} comb[t,e,c]
which cuts FLOPs ~3.4x and never materializes y (B,E,C,O).

Sharding: 8 cores; core k handles batch b=k//2, token half h=k%2 (2048
tokens). Dispatch (which needs the whole batch's tokens) is token-split:
each core computes partial xd for ALL 8 experts from its own 2048
tokens, then 4 pipelined pair-wise AllReduce(add) collectives (256KB
each, groups [[0,1],[2,3],[4,5],[6,7]]) produce the full xd on both
cores of a pair. This halves mask DMA and dispatch matmul work vs
duplicating the dispatch, and the kernel graph stays fully SPMD-uniform
(no core-id-dependent indexing: collective q always carries global
experts {2q, 2q+1}).

Dispatch runs transposed (psum [i, c], rhs = mask streaming N=512) so
each expert uses one PSUM accumulation bank, then 128x128 PE transposes
produce xd in [c, j] orientation for the z-stage. All matmuls run in
bf16 with fp32 PSUM accumulation; inputs are cast/re-laid-out on host.
"""

import numpy as np
import ml_dtypes

BF16 = ml_dtypes.bfloat16

B, T, E, C = 4, 4096, 8, 512
I = 128            # per-expert input features
O = 4096           # out_features
NCORES = 8
TLOC = B * T // NCORES      # 2048 tokens per core
NTTL = TLOC // 128          # 16 local token tiles
NCT = C // 128              # 4 c-blocks
NTC = TLOC // 512           # 4 t-chunks (z stage)
NOT = O // 512              # 8 o-tiles

_CACHE = {}


def _build():
    import concourse.bass as bass
    import concourse.tile as tile
    import concourse.bacc as bacc
    import concourse.mybir as mybir

    f32 = mybir.dt.float32
    bf16 = mybir.dt.bfloat16
    ts = bass.ts

    nc = bacc.Bacc(None, target_bir_lowering=False, debug=False)

    xh = nc.dram_tensor("xh", [E, 128, NTTL, I], bf16, kind="ExternalInput")
    mh = nc.dram_tensor("mh", [E, 128, NTTL, C], bf16, kind="ExternalInput")
    cbt = nc.dram_tensor("cbt", [E, NCT, 128, TLOC], bf16,
                         kind="ExternalInput")
    wf = nc.dram_tensor("wf", [128, E, O], bf16, kind="ExternalInput")
    ident = nc.dram_tensor("ident", [128, 128], bf16, kind="ExternalInput")
    out_d = nc.dram_tensor("out", [TLOC, O], f32, kind="ExternalOutput")

    groups = [[0, 1], [2, 3], [4, 5], [6, 7]]

    with tile.TileContext(nc) as tc:
        with (
            tc.tile_pool(name="persist", bufs=1) as persist,
            tc.tile_pool(name="stream", bufs=1) as stream,
            tc.tile_pool(name="psum", bufs=1, space="PSUM") as psum,
            tc.tile_pool(name="dram", bufs=1, space="DRAM") as dram,
        ):
            wf_sb = persist.tile([128, E, O], bf16, tag="wf")
            id_sb = persist.tile([128, 128], bf16, tag="ident")
            nc.scalar.dma_start(id_sb[:], ident[:])

            cc_in = [dram.tile([2, 128, NCT, 128], bf16, name=f"ccin{q}")
                     for q in range(4)]
            cc_out = [dram.tile([2, 128, NCT, 128], bf16, name=f"ccout{q}")
                      for q in range(4)]

            xd = {}   # e -> reduced xd tile [128c, NCT, 128j] bf16
            zt = {}   # (e, tch) -> z^T tile [128j, 512t] bf16

            def dispatch(e):
                """Partial xd^T for expert e from local tokens; write to
                cc_in[e//2][e%2]."""
                ps_a = psum.tile([128, C], f32, tag="psA", bufs=2,
                                 name=f"psA{e}")
                for q0 in (0, 8):
                    mh_t = stream.tile([128, 8, C], bf16, tag="mh", bufs=4,
                                       name=f"mh{e}_{q0}")
                    nc.sync.dma_start(mh_t[:], mh[e, :, q0:q0 + 8, :])
                    xh_t = stream.tile([128, 8, I], bf16, tag="xh", bufs=4,
                                       name=f"xh{e}_{q0}")
                    nc.scalar.dma_start(xh_t[:], xh[e, :, q0:q0 + 8, :])
                    for i in range(8):
                        tt = q0 + i
                        nc.tensor.matmul(
                            ps_a[:],
                            xh_t[:, i, :],
                            mh_t[:, i, :],
                            start=(tt == 0),
                            stop=(tt == NTTL - 1),
                        )
                xdt = stream.tile([128, C], bf16, tag="xdt", bufs=2,
                                  name=f"xdt{e}")
                nc.vector.tensor_copy(xdt[:], ps_a[:])
                # transpose [i, c] -> [c, j] per 128-block
                xdp = stream.tile([128, NCT, 128], bf16, tag="xdp", bufs=2,
                                  name=f"xdp{e}")
                for cb in range(NCT):
                    ps_t = psum.tile([128, 128], bf16, tag="psT", bufs=2,
                                     name=f"psT{e}_{cb}")
                    nc.tensor.transpose(ps_t[:],
                                        xdt[:, ts(cb, 128)], id_sb[:])
                    nc.vector.tensor_copy(xdp[:, cb, :], ps_t[:])
                nc.gpsimd.dma_start(cc_in[e // 2][e % 2], xdp[:])

            def reduce_pair(q):
                """AllReduce partial xd for experts {2q, 2q+1} and read
                the summed result back into SBUF."""
                nc.gpsimd.collective_compute(
                    "AllReduce",
                    mybir.AluOpType.add,
                    replica_groups=groups,
                    ins=[cc_in[q][:].opt()],
                    outs=[cc_out[q][:].opt()],
                )
                for r in range(2):
                    e = 2 * q + r
                    xr = persist.tile([128, NCT, 128], bf16, tag=f"xd{e}",
                                      name=f"xd{e}")
                    nc.gpsimd.dma_start(xr[:], cc_out[q][r])
                    xd[e] = xr

            def zstage(e, tch):
                cb_t = stream.tile([128, NCT, 512], bf16, tag="cb", bufs=6,
                                   name=f"cb{e}_{tch}")
                for cb in range(NCT):
                    nc.sync.dma_start(cb_t[:, cb, :],
                                      cbt[e, cb, :, ts(tch, 512)])
                ps_z = psum.tile([128, 512], f32, tag="psZT", bufs=2,
                                 name=f"psZ{e}_{tch}")
                for cb in range(NCT):
                    nc.tensor.matmul(
                        ps_z[:],
                        xd[e][:, cb, :],
                        cb_t[:, cb, :],
                        start=(cb == 0),
                        stop=(cb == NCT - 1),
                    )
                z_sb = persist.tile([128, 512], bf16, tag=f"zt{e}_{tch}",
                                    name=f"zt{e}_{tch}")
                nc.vector.tensor_copy(z_sb[:], ps_z[:])
                zt[(e, tch)] = z_sb

            # ---- Own phase: dispatch all 8 experts over local tokens,
            # pair-wise collectives pipelined behind the mask stream.
            # Nothing else competes with the mask/x DMA stream: the last
            # collective (and so the whole combine phase) is gated on it.
            for q in range(4):
                dispatch(2 * q)
                dispatch(2 * q + 1)
                reduce_pair(q)

            # weights + comb tiles stream in the cc latency shadow
            for e in range(E):
                nc.scalar.dma_start(wf_sb[:, e, :], wf[:, e, :])
            # z for pairs 0-2 only needs cc0-cc2 (landed long ago);
            # pair 3 rides right behind cc3.
            for q in range(4):
                for tch in range(NTC):
                    zstage(2 * q, tch)
                    zstage(2 * q + 1, tch)

            # ---- Combine phase (PE-bound) ----
            for tt in range(NTTL):
                tch, m = tt // 4, tt % 4
                out_sb = stream.tile([128, O // 2], f32, tag="out",
                                     bufs=3, name=f"out{tt}")
                for ot in range(NOT):
                    if ot == NOT // 2:
                        nc.scalar.dma_start(
                            out_d[ts(tt, 128), 0:O // 2], out_sb[:])
                        out_sb = stream.tile([128, O // 2], f32,
                                             tag="out", bufs=3,
                                             name=f"out{tt}b")
                    ps_c = psum.tile([128, 512], f32, tag="psC",
                                     bufs=2, name=f"psC{tt}_{ot}")
                    for e in range(E):
                        nc.tensor.matmul(
                            ps_c[:],
                            zt[(e, tch)][:, ts(m, 128)],
                            wf_sb[:, e, ts(ot, 512)],
                            start=(e == 0),
                            stop=(e == E - 1),
                        )
                    nc.vector.tensor_copy(
                        out_sb[:, ts(ot % 4, 512)], ps_c[:])
                nc.scalar.dma_start(
                    out_d[ts(tt, 128), O // 2:O], out_sb[:])

    nc.compile()
    return nc


def _prep_inputs(x, weight, bias, combine_array, dispatch_mask):
    """Host-side cast to bf16 + re-layout for contiguous device DMA."""
    x = np.asarray(x, np.float32)
    weight = np.asarray(weight, np.float32)
    bias = np.asarray(bias, np.float32)
    comb = np.asarray(combine_array, np.float32)
    mask = np.asarray(dispatch_mask, np.float32)

    # xh[b,h]: (E, 128, NTTL, I); [e,p,tt,j] = x[b, h*TLOC+tt*128+p, e, j]
    xh = np.ascontiguousarray(
        x.reshape(B, 2, NTTL, 128, E, I).transpose(0, 1, 4, 3, 2, 5)
    ).astype(BF16)
    # mh[b,h]: (E, 128, NTTL, C)
    mh = np.ascontiguousarray(
        mask.reshape(B, 2, NTTL, 128, E, C).transpose(0, 1, 4, 3, 2, 5)
    ).astype(BF16)
    # cbt[b,h]: (E, NCT, 128, TLOC); [e,cb,p,t] = comb[b, h*TLOC+t, e, cb*128+p]
    cbt = np.ascontiguousarray(
        comb.reshape(B, 2, TLOC, E, NCT, 128).transpose(0, 1, 3, 4, 5, 2)
    ).astype(BF16)
    # wf: (128, E, O); wf[j, e, o] = weight.reshape(E, O, I)[e, o, j]
    wf = np.ascontiguousarray(
        weight.reshape(E, O, I).transpose(2, 0, 1)).astype(BF16)
    # S[b, t] = sum_{e,c} comb[b, t, e, c] -- bias*S added on host in f32
    s = comb.sum(axis=(2, 3))
    idm = np.eye(128, dtype=BF16)

    in_maps = []
    for k in range(NCORES):
        b, h = k // 2, k % 2
        in_maps.append({
            "xh": xh[b, h], "mh": mh[b, h], "cbt": cbt[b, h], "wf": wf,
            "ident": idm,
        })
    return in_maps, s, bias


def kernel(x, weight, bias, combine_array, dispatch_mask):
    from concourse import bass_utils

    if "nc" not in _CACHE:
        _CACHE["nc"] = _build()
    nc = _CACHE["nc"]

    in_maps, s, bias_f = _prep_inputs(
        x, weight, bias, combine_array, dispatch_mask)
    res = bass_utils.run_bass_kernel_spmd(
        nc, in_maps, core_ids=list(range(NCORES)))
    out = np.stack([res.results[k]["out"] for k in range(NCORES)])
    out = out.reshape(B, T, O)
    out += s[:, :, None] * bias_f[None, None, :]
    return out.astype(np.float32)
